# revision 8
# baseline (speedup 1.0000x reference)
"""Multi-head attention (B=16, N=512, H=8, D=128) on 8 trn2 NeuronCores.

Data-parallel over batch: each core handles 2 batches. Per core:
  qT/kT projections in [d, token] layout (fp32r matmuls, N=512 -> 1 cyc/row),
  scores computed transposed sT[m, n] so the attention*V matmul needs no
  transposes and softmax denominators come from PE ones-matmuls.
  exp(s + dist + colmask) is factored as exp(s) * E with E = exp(distT + cm)
  computed once per batch (shared across all 8 heads) -> per-(b,h) elementwise
  work is one ACT exp pass + one DVE bf16 2x multiply pass.
  The v-bias is folded into the output bias on the host (softmax rows sum to
  1 exactly): bo' = bo + Wo^T bv.  Softmax normalization and the final row
  mask fold into the output projection: out = sum_h Wo_h^T (yraw_h * rinvm_h)
  + bo' (x) mask_row, with rinvm = mask / rowsum.
"""

import sys

sys.path.insert(0, "/opt/trn_rl_repo")

import numpy as np
from contextlib import ExitStack

import ml_dtypes
import concourse.bass as bass
import concourse.bacc as bacc
import concourse.tile as tile
from concourse import mybir
from concourse.masks import make_identity

B, N, H, D = 16, 512, 8, 128
NCORES = 8
BPC = B // NCORES  # batches per core
NT = N // 128  # 128-token tiles per batch
F32 = mybir.dt.float32
F32R = mybir.dt.float32r
BF16 = mybir.dt.bfloat16


def r(ap):
    """reinterpret an fp32 AP as float32r for full-rate PE matmuls"""
    return ap.bitcast(F32R)


def bcastP(ap_1d, p):
    """broadcast a 1-d DRAM AP across p partitions"""
    return bass.AP(tensor=ap_1d.tensor, offset=ap_1d.offset, ap=[[0, p]] + ap_1d.ap)


def build_kernel():
    nc = bacc.Bacc("TRN2", target_bir_lowering=False, debug=False)

    x_d = nc.declare_dram_parameter("x_in", [BPC, N, D], F32, isOutput=False).ap()
    dist_d = nc.declare_dram_parameter("dist_in", [BPC, N, N], F32, isOutput=False).ap()
    mask_d = nc.declare_dram_parameter("mask_in", [BPC, N], F32, isOutput=False).ap()
    maskb_d = nc.declare_dram_parameter("maskb_in", [BPC, N], BF16, isOutput=False).ap()
    wq_d = nc.declare_dram_parameter("wq_in", [D, H * D], F32R, isOutput=False).ap()
    wk_d = nc.declare_dram_parameter("wk_in", [D, H * D], F32R, isOutput=False).ap()
    wv_d = nc.declare_dram_parameter("wv_in", [D, H * D], F32R, isOutput=False).ap()
    # Wo pre-arranged on host to [k_within_head, head, d_out], bf16
    wo_d = nc.declare_dram_parameter("wo_in", [D, H, D], BF16, isOutput=False).ap()
    bq_d = nc.declare_dram_parameter("bq_in", [D, H], F32, isOutput=False).ap()
    bk_d = nc.declare_dram_parameter("bk_in", [D, H], F32, isOutput=False).ap()
    bo_d = nc.declare_dram_parameter("bo_in", [D], BF16, isOutput=False).ap()
    y_d = nc.declare_dram_parameter("y_out", [BPC, N, D], F32, isOutput=True).ap()

    rinv_scratch = nc.dram_tensor("rinv_scratch", [BPC, H, N], BF16).ap()

    with tile.TileContext(nc) as tc, ExitStack() as ctx:
        # ---------------- pools ----------------
        consts = ctx.enter_context(tc.tile_pool(name="consts", bufs=1))
        stage = ctx.enter_context(tc.tile_pool(name="stage", bufs=2))
        dnat = ctx.enter_context(tc.tile_pool(name="dnat", bufs=3))
        qkp = ctx.enter_context(tc.tile_pool(name="qkp", bufs=10))
        vpool = ctx.enter_context(tc.tile_pool(name="vpool", bufs=6))
        epool = ctx.enter_context(tc.tile_pool(name="epool", bufs=6))
        xpool = ctx.enter_context(tc.tile_pool(name="xpool", bufs=3))
        ppool = ctx.enter_context(tc.tile_pool(name="ppool", bufs=5))
        ypool = ctx.enter_context(tc.tile_pool(name="ypool", bufs=6))
        rpool = ctx.enter_context(tc.tile_pool(name="rpool", bufs=4))

        # PSUM budget (8 banks): ps_a 2 + ps_y 2 + pst 2 + pso 1 + rs 1
        ps_a = ctx.enter_context(tc.tile_pool(name="ps_a", bufs=2, space="PSUM"))
        ps_y = ctx.enter_context(tc.tile_pool(name="ps_y", bufs=2, space="PSUM"))
        ps_t = ctx.enter_context(tc.tile_pool(name="ps_t", bufs=2, space="PSUM"))
        ps_rs = ctx.enter_context(tc.tile_pool(name="ps_rs", bufs=1, space="PSUM"))

        # ---------------- constants ----------------
        wq_sb = consts.tile([128, H * D], F32R, tag="wq")
        nc.sync.dma_start(out=wq_sb, in_=wq_d)
        wk_sb = consts.tile([128, H * D], F32R, tag="wk")
        nc.sync.dma_start(out=wk_sb, in_=wk_d)
        wv_sb = consts.tile([128, H * D], F32R, tag="wv")
        nc.sync.dma_start(out=wv_sb, in_=wv_d)
        wo_sb = consts.tile([128, H, D], BF16, tag="wo")
        nc.sync.dma_start(out=wo_sb, in_=wo_d)
        bq_sb = consts.tile([128, H], F32, tag="bq")
        nc.sync.dma_start(out=bq_sb, in_=bq_d)
        bk_sb = consts.tile([128, H], F32, tag="bk")
        nc.sync.dma_start(out=bk_sb, in_=bk_d)
        bo_sb = consts.tile([1, D], BF16, tag="bo")
        nc.sync.dma_start(out=bo_sb, in_=bo_d[None, :])
        ident = consts.tile([128, 128], F32, tag="ident")
        make_identity(nc, ident)
        ones_bf = consts.tile([128, 1], BF16, tag="ones")
        nc.vector.memset(ones_bf, 1.0)

        for b in range(BPC):
            # ---------------- masks ----------------
            maskT = stage.tile([128, NT], F32, tag="maskT")
            nc.sync.dma_start(out=maskT, in_=mask_d[b].rearrange("(t p) -> p t", p=128))
            cmT = stage.tile([128, NT], F32, tag="cmT")
            # (mask - 1) * 1e9 : 0 for kept tokens, -1e9 for masked
            nc.vector.tensor_scalar(
                out=cmT, in0=maskT, scalar1=1e9, scalar2=-1e9,
                op0=mybir.AluOpType.mult, op1=mybir.AluOpType.add,
            )
            mask_row = stage.tile([1, N], BF16, tag="mask_row")
            nc.sync.dma_start(out=mask_row, in_=maskb_d[b][None, :])
            mask8 = stage.tile([H, N], F32, tag="mask8")
            nc.sync.dma_start(out=mask8, in_=bcastP(mask_d[b], H))

            # ---------------- x transpose: xT [d, n] ----------------
            x_nat = stage.tile([128, NT, D], F32, tag="x_nat")
            nc.sync.dma_start(out=x_nat, in_=x_d[b].rearrange("(t p) d -> p t d", p=128))
            xT = xpool.tile([128, N], F32R, tag="xT")
            for nt in range(NT):
                pst = ps_t.tile([128, 128], F32, tag="pst")
                nc.tensor.transpose(pst, x_nat[:, nt, :], ident)
                nc.vector.tensor_copy(out=xT[:, nt * 128:(nt + 1) * 128], in_=pst)

            # ---------------- q/k projections -> qT[h], kT[h] [d, n] ----------------
            qT = []
            kT = []
            for h in range(H):
                psq = ps_a.tile([128, N], F32, tag="ps_a", name=f"psq{b}_{h}")
                nc.tensor.matmul(psq, wq_sb[:, h * D:(h + 1) * D], xT)
                qTh = qkp.tile([128, N], F32R, tag="qT", name=f"qT{b}_{h}")
                nc.vector.tensor_scalar_add(out=qTh, in0=psq, scalar1=bq_sb[:, h:h + 1])
                qT.append(qTh)
                psk = ps_a.tile([128, N], F32, tag="ps_a", name=f"psk{b}_{h}")
                nc.tensor.matmul(psk, wk_sb[:, h * D:(h + 1) * D], xT)
                kTh = qkp.tile([128, N], F32R, tag="kT", name=f"kT{b}_{h}")
                nc.vector.tensor_scalar_add(out=kTh, in0=psk, scalar1=bk_sb[:, h:h + 1])
                kT.append(kTh)

            # ---------------- v projection -> v[mt] [m, d_all] (bias folded out) ----------------
            vv = []
            for mt in range(NT):
                vmt = vpool.tile([128, H * D], BF16, tag="vv", name=f"v{b}_{mt}")
                for half in range(2):
                    psv = ps_a.tile([128, N], F32, tag="ps_a", name=f"psv{b}_{mt}_{half}")
                    nc.tensor.matmul(
                        psv,
                        xT[:, mt * 128:(mt + 1) * 128],
                        wv_sb[:, half * 512:(half + 1) * 512],
                    )
                    nc.vector.tensor_copy(
                        out=vmt[:, half * 512:(half + 1) * 512], in_=psv
                    )
                vv.append(vmt)

            # ---------------- dist -> E = exp(distT + colmask) ----------------
            E = [epool.tile([128, N], BF16, tag="E", name=f"E{b}_{mt}") for mt in range(NT)]
            for nt in range(NT):
                dn = dnat.tile([128, N], F32, tag="dnat", name=f"dn{b}_{nt}")
                nc.sync.dma_start(out=dn, in_=dist_d[b, nt * 128:(nt + 1) * 128, :])
                for mt in range(NT):
                    pst = ps_t.tile([128, 128], F32, tag="pst", name=f"pdt{b}_{nt}_{mt}")
                    nc.tensor.transpose(pst, dn[:, mt * 128:(mt + 1) * 128], ident)
                    nc.scalar.activation(
                        out=E[mt][:, nt * 128:(nt + 1) * 128],
                        in_=pst,
                        func=mybir.ActivationFunctionType.Exp,
                        bias=cmT[:, mt:mt + 1],
                    )

            # ---------------- scores + exp + p = exp(s)*E ----------------
            p = [ppool.tile([128, H * N], BF16, tag="p", name=f"p{b}_{mt}") for mt in range(NT)]
            for h in range(H):
                for mt in range(NT):
                    pss = ps_a.tile([128, N], F32, tag="ps_a", name=f"pss{b}_{h}_{mt}")
                    nc.tensor.matmul(pss, kT[h][:, mt * 128:(mt + 1) * 128], qT[h])
                    es = stage.tile([128, N], BF16, tag="exps", bufs=4, name=f"es{b}_{h}_{mt}")
                    nc.scalar.activation(
                        out=es, in_=pss, func=mybir.ActivationFunctionType.Exp
                    )
                    nc.vector.tensor_mul(
                        p[mt][:, h * N:(h + 1) * N], es, E[mt]
                    )

            # ---------------- rowsums + y ----------------
            rs8 = stage.tile([H, N], F32, tag="rs8")
            yraws = []
            for h in range(H):
                prs = ps_rs.tile([1, N], F32, tag="rs", name=f"prs{b}_{h}")
                for mt in range(NT):
                    nc.tensor.matmul(
                        prs, ones_bf, p[mt][:, h * N:(h + 1) * N],
                        start=(mt == 0), stop=(mt == NT - 1),
                    )
                rs_h = stage.tile([1, N], F32, tag="rs_h", bufs=3, name=f"rs{b}_{h}")
                nc.vector.tensor_copy(out=rs_h, in_=prs)
                # engines can't write at partition offset h; DMA can
                nc.sync.dma_start(out=rs8[h:h + 1, :], in_=rs_h)
                py = ps_y.tile([128, N], F32, tag="ps_y", name=f"py{b}_{h}")
                for mt in range(NT):
                    nc.tensor.matmul(
                        py,
                        vv[mt][:, h * D:(h + 1) * D],
                        p[mt][:, h * N:(h + 1) * N],
                        start=(mt == 0), stop=(mt == NT - 1),
                    )
                yraw = ypool.tile([128, N], BF16, tag="yraw", name=f"yraw{b}_{h}")
                nc.vector.tensor_copy(out=yraw, in_=py)
                yraws.append(yraw)

            # ---------------- rinvm = mask / rowsum, broadcast via DRAM ----------------
            rinv = stage.tile([H, N], F32, tag="rinv")
            nc.vector.reciprocal_approx_fast(out=rinv, in_=rs8)
            rinvm = stage.tile([H, N], BF16, tag="rinvm")
            nc.vector.tensor_mul(rinvm, rinv, mask8)
            nc.sync.dma_start(out=rinv_scratch[b], in_=rinvm)

            # ---------------- normalize + output projection (transposed) ----------------
            pso = ps_t.tile([128, N], F32, tag="pso", bufs=1, name=f"pso{b}")
            for h in range(H):
                rB = rpool.tile([128, N], BF16, tag="rB", name=f"rB{b}_{h}")
                nc.sync.dma_start(out=rB, in_=bcastP(rinv_scratch[b, h], 128))
                yTn = ypool.tile([128, N], BF16, tag="yTn", name=f"yTn{b}_{h}")
                nc.vector.tensor_mul(yTn, yraws[h], rB)
                nc.tensor.matmul(
                    pso, wo_sb[:, h, :], yTn,
                    start=(h == 0), stop=False,
                )
            nc.tensor.matmul(pso, bo_sb, mask_row, start=False, stop=True)
            oT = stage.tile([128, N], F32, tag="oT")
            nc.vector.tensor_copy(out=oT, in_=pso)

            # ---------------- transpose back to [n, d] and store ----------------
            o_nat = stage.tile([128, NT, D], F32, tag="o_nat")
            for nt in range(NT):
                pst = ps_t.tile([128, 128], F32, tag="pst", name=f"pot{b}_{nt}")
                nc.tensor.transpose(pst, oT[:, nt * 128:(nt + 1) * 128], ident)
                nc.vector.tensor_copy(out=o_nat[:, nt, :], in_=pst)
            nc.sync.dma_start(
                out=y_d[b].rearrange("(t p) d -> p t d", p=128), in_=o_nat
            )

    nc.compile()
    return nc


_NC_CACHE = None


def _get_nc():
    global _NC_CACHE
    if _NC_CACHE is None:
        _NC_CACHE = build_kernel()
    return _NC_CACHE


def kernel(x, dist, mask, Wq, bq, Wk, bk, Wv, bv, Wo, bo, **kw):
    from concourse.bass_utils import run_bass_kernel_spmd

    x = np.ascontiguousarray(np.asarray(x, dtype=np.float32))
    dist = np.ascontiguousarray(np.asarray(dist, dtype=np.float32))
    mask = np.ascontiguousarray(np.asarray(mask, dtype=np.float32))
    Wq = np.asarray(Wq, np.float32)
    Wk = np.asarray(Wk, np.float32)
    Wv = np.asarray(Wv, np.float32)
    Wo = np.asarray(Wo, np.float32)
    bq = np.asarray(bq, np.float32)
    bk = np.asarray(bk, np.float32)
    bv = np.asarray(bv, np.float32)
    bo = np.asarray(bo, np.float32)

    scale = np.float32(D) ** np.float32(-0.5)
    wq_s = np.ascontiguousarray(Wq * scale)
    bq_s = np.ascontiguousarray((bq * scale).reshape(H, D).T)  # [d, h]
    bk_r = np.ascontiguousarray(bk.reshape(H, D).T)
    # Wo [H*D, D] -> [k_within_head, head, d_out]
    wo_r = np.ascontiguousarray(Wo.reshape(H, D, D).transpose(1, 0, 2))
    wo_bf = wo_r.astype(ml_dtypes.bfloat16)
    # fold the v-bias through the output projection (softmax rows sum to 1)
    bo_eff = (bo + bv @ Wo).astype(np.float32)
    bo_bf = bo_eff.astype(ml_dtypes.bfloat16)
    mask_bf = mask.astype(ml_dtypes.bfloat16)

    nc = _get_nc()
    in_maps = []
    for c in range(NCORES):
        sl = slice(c * BPC, (c + 1) * BPC)
        in_maps.append(
            {
                "x_in": x[sl],
                "dist_in": dist[sl],
                "mask_in": mask[sl],
                "maskb_in": mask_bf[sl],
                "wq_in": wq_s,
                "wk_in": Wk,
                "wv_in": Wv,
                "wo_in": wo_bf,
                "bq_in": bq_s,
                "bk_in": bk_r,
                "bo_in": bo_bf,
            }
        )
    res = run_bass_kernel_spmd(nc, in_maps, core_ids=list(range(NCORES)), **kw)
    global LAST_RESULT
    LAST_RESULT = res
    out = np.concatenate([res.results[c]["y_out"] for c in range(NCORES)], axis=0)
    return out


LAST_RESULT = None


if __name__ == "__main__":
    nc = build_kernel()
    print("kernel built ok")
